# revision 6
# baseline (speedup 1.0000x reference)
"""8-core data-parallel fused attention kernel for TRN2 (Bass/Tile).

Problem: B=8, N=1024 (32x32 grid), DIM=1024, 16 heads x 64, axial RoPE on
first 32 channels of each head, softmax attention, output projection.

Sharding: pure data-parallel -- core b computes batch element b end-to-end.
No collectives.

Device layout tricks (all host-prepped):
- All matmuls in bf16 (PSUM accumulates f32). rel-err budget 2e-2.
- QKV computed "transposed" (features on partitions, tokens free) so that
  scores^T[k,q] matmuls need no on-chip transposes.
- rotate_half is folded into extra matmul columns: q~ = x @ (P Wq)^T where P
  is the pair-swap/negate permutation, applied to Wq rows host-side. RoPE is
  then elementwise: q_rope = q*cos + q~*sin (no partner-lane access).
- Q/K rot and pass channels live in separate SBUF tensors; scores contract
  d in two K=32 matmuls accumulated in PSUM, 2-way row-group packed.
- V gets a ones-column appended, so attn@V also yields softmax denominators
  (row 64 of the output) for free. exp() skips max-subtraction (scores are
  O(1) by construction: weights scaled 0.02).
- Softmax normalization: reciprocal on DVE + DMA partition-broadcast + one
  tensor_tensor multiply folded into the PSUM->SBUF drain.
"""

import os
import sys

for _p in ("/opt/trn_rl_repo",):
    if os.path.isdir(_p) and _p not in sys.path:
        sys.path.insert(0, _p)

import numpy as np
import ml_dtypes

import concourse.bass as bass
import concourse.bacc as bacc
import concourse.mybir as mybir
import concourse.tile as tile
from concourse.bass_utils import run_bass_kernel_spmd

P = 128
NTOK = 1024
DIM = 1024
HEADS = 16
HD = 64
ROT = 32
QT = 512          # free-dim tile for matmuls (one PSUM bank of f32)
NQ = NTOK // QT   # 2
BF = mybir.dt.bfloat16
F32 = mybir.dt.float32
AL = mybir.AluOpType
AF = mybir.ActivationFunctionType

LAST_RESULT = None
_BUILT = None


# ---------------------------------------------------------------- host prep

def _axial_tables():
    """cos/sin[t, d] for t=0..1023 (t=h*32+w), d=0..31, exactly as reference."""
    rot_half = 8
    base = np.linspace(1.0, 512.0, rot_half) * np.pi          # (8,)
    th = np.linspace(-1.0, 1.0, 32)[:, None] * base[None, :]  # (32, 8)
    fh = np.repeat(th, 2, axis=-1)                            # (32, 16)
    freqs = np.zeros((32, 32, ROT))
    freqs[:, :, :16] = fh[:, None, :]                         # H-axis channels
    freqs[:, :, 16:] = fh[None, :, :]                         # W-axis channels
    f = freqs.reshape(NTOK, ROT)
    return np.cos(f).astype(np.float32), np.sin(f).astype(np.float32)


def _tilde(W):
    """Row permutation+negation implementing rotate_half: rows h*64+d."""
    W4 = W.reshape(HEADS, HD // 2, 2, DIM)
    Wt = np.empty_like(W4)
    Wt[:, :, 0] = -W4[:, :, 1]
    Wt[:, :, 1] = W4[:, :, 0]
    return Wt.reshape(HEADS * HD, DIM)


def _sel(W, lo, hi):
    """Quad-major row select: for m(4) for hl(4) for d in [lo,hi)."""
    idx = [(4 * m + hl) * HD + d
           for m in range(4) for hl in range(4) for d in range(lo, hi)]
    return W[idx]


def _prep_weights(Wqkv, Wproj, bproj):
    Wq, Wk, Wv = Wqkv[0:DIM], Wqkv[DIM:2 * DIM], Wqkv[2 * DIM:3 * DIM]
    Wqt, Wkt = _tilde(Wq), _tilde(Wk)
    # column groups: 0 Qrot, 1 Q~rot, 2 Krot, 3 K~rot, 4 Qpass, 5 Kpass
    wqk = np.concatenate([
        _sel(Wq, 0, ROT), _sel(Wqt, 0, ROT),
        _sel(Wk, 0, ROT), _sel(Wkt, 0, ROT),
        _sel(Wq, ROT, HD), _sel(Wk, ROT, HD),
    ], axis=0)                                # (3072, 1024)
    cos_td, sin_td = _axial_tables()          # (1024, 32)
    cosR = np.tile(cos_td.T, (4, 1))          # (128, 1024): row hl*32+d
    sinR = np.tile(sin_td.T, (4, 1))
    biasT = bproj.reshape(8, P).T.copy()      # (128, 8)
    bf = ml_dtypes.bfloat16
    return {
        "wqk": np.ascontiguousarray(wqk.T).astype(bf),     # (1024, 3072)
        "wv": np.ascontiguousarray(Wv.T).astype(bf),       # (1024, 1024)
        "wp": np.ascontiguousarray(Wproj.T).astype(bf),    # (1024, 1024)
        "cosr": np.ascontiguousarray(cosR.astype(np.float32)),
        "sinr": np.ascontiguousarray(sinR.astype(np.float32)),
        "biasT": np.ascontiguousarray(biasT.astype(np.float32)),
    }


# ------------------------------------------------------------- bass builder

def _build():
    nc = bacc.Bacc()
    xT_e = nc.declare_dram_parameter("xT", [DIM, NTOK], BF, isOutput=False)
    wqk_e = nc.declare_dram_parameter("wqk", [DIM, 3 * DIM], BF, isOutput=False)
    wv_e = nc.declare_dram_parameter("wv", [DIM, DIM], BF, isOutput=False)
    wp_e = nc.declare_dram_parameter("wp", [DIM, DIM], BF, isOutput=False)
    cos_e = nc.declare_dram_parameter("cosr", [P, NTOK], F32, isOutput=False)
    sin_e = nc.declare_dram_parameter("sinr", [P, NTOK], F32, isOutput=False)
    b_e = nc.declare_dram_parameter("biasT", [P, 8], F32, isOutput=False)
    out_e = nc.declare_dram_parameter("out", [DIM, NTOK], F32, isOutput=True)

    with tile.TileContext(nc) as tc:
        with (
            tc.tile_pool(name="persist", bufs=1) as persist,
            tc.tile_pool(name="work", bufs=2) as work,
            tc.tile_pool(name="work3", bufs=3) as work3,
            tc.tile_pool(name="ps_sc", bufs=2, space="PSUM") as ps_sc_pool,
            tc.tile_pool(name="ps_av", bufs=2, space="PSUM") as ps_av_pool,
            tc.tile_pool(name="ps_mm", bufs=2, space="PSUM") as ps_mm_pool,
            tc.tile_pool(name="dramsc", bufs=4, space="DRAM") as dram_pool,
        ):
            xT = persist.tile([P, 8, NTOK], BF)
            wqk = persist.tile([P, 8, 3 * DIM], BF)
            wv = persist.tile([P, 8, DIM], BF)
            wp = persist.tile([P, 8, DIM], BF)
            cosr = persist.tile([P, NTOK], F32)
            sinr = persist.tile([P, NTOK], F32)
            biasT = persist.tile([P, 8], F32)
            # rope'd QK: partition hl*32+d, chunk = quad index m
            qrot = persist.tile([P, 4, NTOK], BF)
            qpas = persist.tile([P, 4, NTOK], BF)
            krot = persist.tile([P, 4, NTOK], BF)
            kpas = persist.tile([P, 4, NTOK], BF)
            # V with ones column: [k-token partitions, kc, head, 65]
            vaug = persist.tile([P, 8, HEADS * (HD + 1)], BF)
            # attention out, transposed: partition 64*(h%2)+d, chunk h//2
            outT = persist.tile([P, 8, NTOK], BF)

            vaug_r = vaug.rearrange("p n (h e) -> p n h e", e=HD + 1)

            # ---------------- input DMAs (xT/wv first: V phase runs while
            # the 6MB wqk load streams in)
            for cc in range(8):
                nc.sync.dma_start(out=xT[:, cc, :], in_=xT_e[cc * P:(cc + 1) * P, :])
            for cc in range(8):
                nc.sync.dma_start(out=wv[:, cc, :], in_=wv_e[cc * P:(cc + 1) * P, :])
            nc.sync.dma_start(out=cosr[:], in_=cos_e[:, :])
            nc.sync.dma_start(out=sinr[:], in_=sin_e[:, :])
            nc.sync.dma_start(out=biasT[:], in_=b_e[:, :])
            for cc in range(8):
                nc.sync.dma_start(out=wqk[:, cc, :], in_=wqk_e[cc * P:(cc + 1) * P, :])
            for cc in range(8):
                nc.sync.dma_start(out=wp[:, cc, :], in_=wp_e[cc * P:(cc + 1) * P, :])

            # ---------------- V = x @ Wv^T, natural orientation
            def v_phase():
                for tt in range(8):
                    for g in range(2):
                        pt = ps_mm_pool.tile([P, QT], F32, tag="ps_mm")
                        for cc in range(8):
                            nc.tensor.matmul(
                                pt[:],
                                xT[:, cc, tt * P:(tt + 1) * P],
                                wv[:, cc, g * QT:(g + 1) * QT],
                                start=(cc == 0), stop=(cc == 7))
                        nc.vector.tensor_copy(
                            vaug_r[:, tt, g * 8:(g + 1) * 8, 0:HD],
                            pt[:].rearrange("p (h d) -> p h d", d=HD))
                    nc.vector.memset(vaug_r[:, tt, :, HD:HD + 1], 1.0)

            # ---------------- QKV^T for one quad of 4 heads + RoPE epilogue
            def qkv_quad(m):
                for t2 in range(NQ):
                    ts_ = slice(t2 * QT, (t2 + 1) * QT)

                    def mm_group(gi):
                        pt = ps_mm_pool.tile([P, QT], F32, tag="ps_mm")
                        col0 = (gi * 4 + m) * P
                        for cc in range(8):
                            nc.tensor.matmul(
                                pt[:],
                                wqk[:, cc, col0:col0 + P],
                                xT[:, cc, ts_],
                                start=(cc == 0), stop=(cc == 7))
                        return pt

                    for rot_g, til_g, dst in ((0, 1, qrot), (2, 3, krot)):
                        pr = mm_group(rot_g)
                        ptl = mm_group(til_g)
                        t1 = work.tile([P, QT], BF, tag="t1")
                        t2b = work.tile([P, QT], BF, tag="t2")
                        nc.vector.tensor_tensor(t1[:], pr[:], cosr[:, ts_], op=AL.mult)
                        nc.vector.tensor_tensor(t2b[:], ptl[:], sinr[:, ts_], op=AL.mult)
                        nc.vector.tensor_add(dst[:, m, ts_], t1[:], t2b[:])
                    for pas_g, dst in ((4, qpas), (5, kpas)):
                        pp = mm_group(pas_g)
                        nc.vector.tensor_copy(dst[:, m, ts_], pp[:])

            # ---------------- attention for one quad (2 head-pairs)
            def attn_quad(m):
                for p2 in range(2):
                    h0 = 4 * m + 2 * p2
                    h1 = h0 + 1
                    pb0 = (2 * p2) * 32
                    pb1 = pb0 + 32
                    for qt in range(NQ):
                        qs = slice(qt * QT, (qt + 1) * QT)
                        po0 = ps_av_pool.tile([HD + 1, QT], F32, tag="ps_o")
                        po1 = ps_av_pool.tile([HD + 1, QT], F32, tag="ps_o")
                        for kc in range(8):
                            ks = slice(kc * P, (kc + 1) * P)
                            pss = ps_sc_pool.tile([P, 2 * QT], F32, tag="ps_s")
                            # scores^T tiles for both heads, 2-way row packed
                            nc.tensor.matmul(
                                pss[:, 0:QT],
                                krot[pb0:pb0 + 32, m, ks], qrot[pb0:pb0 + 32, m, qs],
                                start=True, stop=False, tile_position=(pb0, 0))
                            nc.tensor.matmul(
                                pss[:, QT:2 * QT],
                                krot[pb1:pb1 + 32, m, ks], qrot[pb1:pb1 + 32, m, qs],
                                start=True, stop=False, tile_position=(pb1, 0))
                            nc.tensor.matmul(
                                pss[:, 0:QT],
                                kpas[pb0:pb0 + 32, m, ks], qpas[pb0:pb0 + 32, m, qs],
                                start=False, stop=True, tile_position=(pb0, 0))
                            nc.tensor.matmul(
                                pss[:, QT:2 * QT],
                                kpas[pb1:pb1 + 32, m, ks], qpas[pb1:pb1 + 32, m, qs],
                                start=False, stop=True, tile_position=(pb1, 0))
                            aT = work3.tile([P, 2 * QT], BF, tag="aT")
                            nc.scalar.activation(aT[:], pss[:], AF.Exp, scale=0.125)
                            nc.tensor.matmul(
                                po0[:], vaug_r[:, kc, h0, :], aT[:, 0:QT],
                                start=(kc == 0), stop=(kc == 7))
                            nc.tensor.matmul(
                                po1[:], vaug_r[:, kc, h1, :], aT[:, QT:2 * QT],
                                start=(kc == 0), stop=(kc == 7))
                        for po, h in ((po0, h0), (po1, h1)):
                            den = work.tile([HD + 1, QT], F32, tag="den")
                            nc.vector.reciprocal(den[HD:HD + 1, :], po[HD:HD + 1, :])
                            dsc = dram_pool.tile([1, QT], F32, tag="dsc")
                            nc.sync.dma_start(out=dsc[:], in_=den[HD:HD + 1, :])
                            bc = work.tile([HD, QT], F32, tag="bc")
                            nc.sync.dma_start(
                                out=bc[:], in_=dsc[:].to_broadcast((HD, QT)))
                            pbase = HD * (h % 2)
                            nc.vector.tensor_tensor(
                                outT[pbase:pbase + HD, h // 2, qs],
                                po[0:HD, :], bc[:], op=AL.mult)

            # ---------------- output projection + bias
            def proj():
                for ot in range(8):
                    os_ = slice(ot * P, (ot + 1) * P)
                    for qt in range(NQ):
                        qs = slice(qt * QT, (qt + 1) * QT)
                        pt = ps_mm_pool.tile([P, QT], F32, tag="ps_mm")
                        for cc in range(8):
                            nc.tensor.matmul(
                                pt[:], wp[:, cc, os_], outT[:, cc, qs],
                                start=(cc == 0), stop=(cc == 7))
                        ys = work.tile([P, QT], F32, tag="ys")
                        nc.vector.tensor_scalar_add(ys[:], pt[:], biasT[:, ot:ot + 1])
                        nc.sync.dma_start(out=out_e[os_, qs], in_=ys[:])

            v_phase()
            qkv_quad(0)
            qkv_quad(1)
            attn_quad(0)
            qkv_quad(2)
            attn_quad(1)
            qkv_quad(3)
            attn_quad(2)
            attn_quad(3)
            proj()

    nc.compile()
    return nc


def _get_nc():
    global _BUILT
    if _BUILT is None:
        _BUILT = _build()
    return _BUILT


# ------------------------------------------------- tracing support (axon)

def _ensure_trace_hooks():
    """Register the NTFF profile hook that the bare agent image's antenv
    stub lacks, and neuter the artifact upload (no bucket in-container)."""
    import types
    import concourse.bass_utils as bu

    bu.upload_artifacts = lambda tmpdir: f"local:{tmpdir}"
    try:
        from antenv.axon_hooks import get_axon_ntff_profile_hook  # noqa: F401
        return
    except ImportError:
        pass
    mod = types.ModuleType("antenv.axon_hooks")
    _state = {"hook": None}
    mod.set_axon_ntff_profile_hook = lambda h: _state.__setitem__("hook", h)
    mod.get_axon_ntff_profile_hook = lambda: _state["hook"]
    import antenv
    sys.modules["antenv.axon_hooks"] = mod
    antenv.axon_hooks = mod
    try:
        from trn_agent_boot.trn_boot import _ntff_profile_via_ctypes
        hook = _ntff_profile_via_ctypes("/opt/axon/libaxon_pjrt.so")
        if hook is not None:
            mod.set_axon_ntff_profile_hook(hook)
    except Exception as e:  # pragma: no cover
        print(f"NTFF hook install failed: {e!r}")


# ----------------------------------------------------------------- kernel()

def kernel(x, Wqkv, Wproj, bproj):
    global LAST_RESULT
    x = np.asarray(x, np.float32)
    Wqkv = np.asarray(Wqkv, np.float32)
    Wproj = np.asarray(Wproj, np.float32)
    bproj = np.asarray(bproj, np.float32)
    B = x.shape[0]

    base = _prep_weights(Wqkv, Wproj, bproj)
    bf = ml_dtypes.bfloat16
    in_maps = [
        dict(base, xT=np.ascontiguousarray(x[b].T).astype(bf)) for b in range(B)
    ]
    nc = _get_nc()
    trace = bool(os.environ.get("KBENCH_TRACE"))
    if trace:
        _ensure_trace_hooks()
    res = run_bass_kernel_spmd(
        nc, in_maps, core_ids=list(range(B)), trace=trace)
    LAST_RESULT = res
    out = np.stack([np.asarray(res.results[b]["out"]).T for b in range(B)])
    return np.ascontiguousarray(out.astype(np.float32))


# revision 10
# speedup vs baseline: 1.0484x; 1.0484x over previous
"""8-core data-parallel fused attention kernel for TRN2 (Bass/Tile).

Problem: B=8, N=1024 (32x32 grid), DIM=1024, 16 heads x 64, axial RoPE on
first 32 channels of each head, softmax attention, output projection.

Sharding: pure data-parallel -- core b computes batch element b end-to-end.
No collectives.

Device layout tricks (all host-prepped):
- All matmuls in bf16 (PSUM accumulates f32). rel-err budget 2e-2.
- QKV computed "transposed" (features on partitions, tokens free) so that
  scores^T[k,q] matmuls need no on-chip transposes.
- rotate_half is folded into extra matmul columns: q~ = x @ (P Wq)^T where P
  is the pair-swap/negate permutation, applied to Wq rows host-side. RoPE is
  then elementwise: q_rope = q*cos + q~*sin (no partner-lane access).
- Q/K rot and pass channels live in separate SBUF tensors; scores contract
  d in two K=32 matmuls accumulated in PSUM, 2-way row-group packed.
- V gets a ones-column appended, so attn@V also yields softmax denominators
  (row 64 of the output) for free. exp() skips max-subtraction (scores are
  O(1) by construction: weights scaled 0.02).
- Softmax normalization: reciprocal on DVE + DMA partition-broadcast + one
  tensor_tensor multiply folded into the PSUM->SBUF drain.
"""

import os
import sys

for _p in ("/opt/trn_rl_repo",):
    if os.path.isdir(_p) and _p not in sys.path:
        sys.path.insert(0, _p)

import numpy as np
import ml_dtypes

import concourse.bass as bass
import concourse.bacc as bacc
import concourse.mybir as mybir
import concourse.tile as tile
from concourse.bass_utils import run_bass_kernel_spmd

P = 128
NTOK = 1024
DIM = 1024
HEADS = 16
HD = 64
ROT = 32
QT = 512          # free-dim tile for matmuls (one PSUM bank of f32)
NQ = NTOK // QT   # 2
BF = mybir.dt.bfloat16
F32 = mybir.dt.float32
AL = mybir.AluOpType
AF = mybir.ActivationFunctionType

LAST_RESULT = None
_BUILT = None


# ---------------------------------------------------------------- host prep

def _axial_tables():
    """cos/sin[t, d] for t=0..1023 (t=h*32+w), d=0..31, exactly as reference."""
    rot_half = 8
    base = np.linspace(1.0, 512.0, rot_half) * np.pi          # (8,)
    th = np.linspace(-1.0, 1.0, 32)[:, None] * base[None, :]  # (32, 8)
    fh = np.repeat(th, 2, axis=-1)                            # (32, 16)
    freqs = np.zeros((32, 32, ROT))
    freqs[:, :, :16] = fh[:, None, :]                         # H-axis channels
    freqs[:, :, 16:] = fh[None, :, :]                         # W-axis channels
    f = freqs.reshape(NTOK, ROT)
    return np.cos(f).astype(np.float32), np.sin(f).astype(np.float32)


def _tilde(W):
    """Row permutation+negation implementing rotate_half: rows h*64+d."""
    W4 = W.reshape(HEADS, HD // 2, 2, DIM)
    Wt = np.empty_like(W4)
    Wt[:, :, 0] = -W4[:, :, 1]
    Wt[:, :, 1] = W4[:, :, 0]
    return Wt.reshape(HEADS * HD, DIM)


def _sel(W, lo, hi):
    """Quad-major row select: for m(4) for hl(4) for d in [lo,hi)."""
    idx = [(4 * m + hl) * HD + d
           for m in range(4) for hl in range(4) for d in range(lo, hi)]
    return W[idx]


def _prep_weights(Wqkv, Wproj, bproj):
    Wq, Wk, Wv = Wqkv[0:DIM], Wqkv[DIM:2 * DIM], Wqkv[2 * DIM:3 * DIM]
    Wqt, Wkt = _tilde(Wq), _tilde(Wk)
    # column groups: 0 Qrot, 1 Q~rot, 2 Krot, 3 K~rot, 4 Qpass, 5 Kpass
    # quad-major: for each quad m, 6 groups of 128 rows
    # (Qrot, Q~rot, Krot, K~rot, Qpass, Kpass), each covering heads 4m..4m+3
    grp = [
        _sel(Wq, 0, ROT), _sel(Wqt, 0, ROT),
        _sel(Wk, 0, ROT), _sel(Wkt, 0, ROT),
        _sel(Wq, ROT, HD), _sel(Wk, ROT, HD),
    ]
    wqk = np.concatenate(
        [g[m * P:(m + 1) * P] for m in range(4) for g in grp], axis=0)
    cos_td, sin_td = _axial_tables()          # (1024, 32)
    cosR = np.tile(cos_td.T, (4, 1))          # (128, 1024): row hl*32+d
    sinR = np.tile(sin_td.T, (4, 1))
    biasT = bproj.reshape(8, P).T.copy()      # (128, 8)
    bf = ml_dtypes.bfloat16
    return {
        "wqk": np.ascontiguousarray(wqk.T).astype(bf),     # (1024, 3072)
        "wv": np.ascontiguousarray(Wv.T).astype(bf),       # (1024, 1024)
        "wp": np.ascontiguousarray(Wproj.T).astype(bf),    # (1024, 1024)
        "cosr": np.ascontiguousarray(cosR.astype(np.float32)),
        "sinr": np.ascontiguousarray(sinR.astype(np.float32)),
        "biasT": np.ascontiguousarray(biasT.astype(np.float32)),
    }


# ------------------------------------------------------------- bass builder

def _build():
    nc = bacc.Bacc()
    xT_e = nc.declare_dram_parameter("xT", [DIM, NTOK], BF, isOutput=False)
    wqk_e = nc.declare_dram_parameter("wqk", [DIM, 3 * DIM], BF, isOutput=False)
    wv_e = nc.declare_dram_parameter("wv", [DIM, DIM], BF, isOutput=False)
    wp_e = nc.declare_dram_parameter("wp", [DIM, DIM], BF, isOutput=False)
    cos_e = nc.declare_dram_parameter("cosr", [P, NTOK], F32, isOutput=False)
    sin_e = nc.declare_dram_parameter("sinr", [P, NTOK], F32, isOutput=False)
    b_e = nc.declare_dram_parameter("biasT", [P, 8], F32, isOutput=False)
    out_e = nc.declare_dram_parameter("out", [DIM, NTOK], F32, isOutput=True)

    with tile.TileContext(nc) as tc:
        with (
            tc.tile_pool(name="persist", bufs=1) as persist,
            tc.tile_pool(name="work", bufs=2) as work,
            tc.tile_pool(name="work3", bufs=3) as work3,
            tc.tile_pool(name="ps_sc", bufs=2, space="PSUM") as ps_sc_pool,
            tc.tile_pool(name="ps_av", bufs=2, space="PSUM") as ps_av_pool,
            tc.tile_pool(name="ps_mm", bufs=2, space="PSUM") as ps_mm_pool,
            tc.tile_pool(name="dramsc", bufs=4, space="DRAM") as dram_pool,
        ):
            xT = persist.tile([P, 8, NTOK], BF)
            wqk = persist.tile([P, 8, 3 * DIM], BF)
            wv = persist.tile([P, 8, DIM], BF)
            wp = persist.tile([P, 8, DIM], BF)
            cosr = persist.tile([P, NTOK], F32)
            sinr = persist.tile([P, NTOK], F32)
            biasT = persist.tile([P, 8], F32)
            # rope'd QK: partition hl*32+d, chunk = quad index m
            qrot = persist.tile([P, 4, NTOK], BF)
            qpas = persist.tile([P, 4, NTOK], BF)
            krot = persist.tile([P, 4, NTOK], BF)
            kpas = persist.tile([P, 4, NTOK], BF)
            # V with ones column: [k-token partitions, kc, head, 65]
            vaug = persist.tile([P, 8, HEADS * (HD + 1)], BF)
            # attention out, transposed: partition 64*(h%2)+d, chunk h//2
            outT = persist.tile([P, 8, NTOK], BF)

            vaug_r = vaug.rearrange("p n (h e) -> p n h e", e=HD + 1)

            # ---------------- input DMAs. Order matters: xT/wv feed the V
            # phase, wqk quad 0 feeds qkv(0); the rest streams in behind.
            for cc in range(8):
                nc.sync.dma_start(out=xT[:, cc, :], in_=xT_e[cc * P:(cc + 1) * P, :])
            for cc in range(8):
                nc.sync.dma_start(out=wv[:, cc, :], in_=wv_e[cc * P:(cc + 1) * P, :])
            nc.sync.dma_start(out=cosr[:], in_=cos_e[:, :])
            nc.sync.dma_start(out=sinr[:], in_=sin_e[:, :])
            nc.sync.dma_start(out=biasT[:], in_=b_e[:, :])
            # wqk columns are quad-major: quad m occupies cols [m*768, (m+1)*768)
            for m in range(4):
                for cc in range(8):
                    nc.sync.dma_start(
                        out=wqk[:, cc, m * 768:(m + 1) * 768],
                        in_=wqk_e[cc * P:(cc + 1) * P, m * 768:(m + 1) * 768])
            for cc in range(8):
                nc.sync.dma_start(out=wp[:, cc, :], in_=wp_e[cc * P:(cc + 1) * P, :])

            # ---------------- V = x @ Wv^T, natural orientation
            def v_units():
                for tt in range(8):
                    for g in range(2):
                        pt = ps_mm_pool.tile([P, QT], F32, tag="ps_mm")
                        for cc in range(8):
                            nc.tensor.matmul(
                                pt[:],
                                xT[:, cc, tt * P:(tt + 1) * P],
                                wv[:, cc, g * QT:(g + 1) * QT],
                                start=(cc == 0), stop=(cc == 7))
                        nc.vector.tensor_copy(
                            vaug_r[:, tt, g * 8:(g + 1) * 8, 0:HD],
                            pt[:].rearrange("p (h d) -> p h d", d=HD))
                        yield
                    nc.vector.memset(vaug_r[:, tt, :, HD:HD + 1], 1.0)

            # ---------------- QKV^T for one quad of 4 heads + RoPE epilogue
            def qkv_units(m):
                def mm_group(gi, t2):
                    pt = ps_mm_pool.tile([P, QT], F32, tag="ps_mm")
                    col0 = m * 768 + gi * P
                    ts_ = slice(t2 * QT, (t2 + 1) * QT)
                    for cc in range(8):
                        nc.tensor.matmul(
                            pt[:],
                            wqk[:, cc, col0:col0 + P],
                            xT[:, cc, ts_],
                            start=(cc == 0), stop=(cc == 7))
                    return pt

                for t2 in range(NQ):
                    ts_ = slice(t2 * QT, (t2 + 1) * QT)
                    for rot_g, til_g, dst in ((0, 1, qrot), (2, 3, krot)):
                        pr = mm_group(rot_g, t2)
                        yield
                        ptl = mm_group(til_g, t2)
                        t1 = work.tile([P, QT], BF, tag="t1")
                        t2b = work.tile([P, QT], BF, tag="t2")
                        nc.vector.tensor_tensor(t1[:], pr[:], cosr[:, ts_], op=AL.mult)
                        nc.vector.tensor_tensor(t2b[:], ptl[:], sinr[:, ts_], op=AL.mult)
                        nc.vector.tensor_add(dst[:, m, ts_], t1[:], t2b[:])
                        yield
                    for pas_g, dst in ((4, qpas), (5, kpas)):
                        pp = mm_group(pas_g, t2)
                        nc.vector.tensor_copy(dst[:, m, ts_], pp[:])
                        yield

            # ---------------- attention for one quad (2 head-pairs)
            def attn_units(m):
                for p2 in range(2):
                    h0 = 4 * m + 2 * p2
                    h1 = h0 + 1
                    pb0 = (2 * p2) * 32
                    pb1 = pb0 + 32
                    for qt in range(NQ):
                        qs = slice(qt * QT, (qt + 1) * QT)
                        po0 = ps_av_pool.tile([HD + 1, QT], F32, tag="ps_o")
                        po1 = ps_av_pool.tile([HD + 1, QT], F32, tag="ps_o")
                        for kc in range(8):
                            ks = slice(kc * P, (kc + 1) * P)
                            pss = ps_sc_pool.tile([P, 2 * QT], F32, tag="ps_s")
                            nc.tensor.matmul(
                                pss[:, 0:QT],
                                krot[pb0:pb0 + 32, m, ks], qrot[pb0:pb0 + 32, m, qs],
                                start=True, stop=False, tile_position=(pb0, 0))
                            nc.tensor.matmul(
                                pss[:, QT:2 * QT],
                                krot[pb1:pb1 + 32, m, ks], qrot[pb1:pb1 + 32, m, qs],
                                start=True, stop=False, tile_position=(pb1, 0))
                            nc.tensor.matmul(
                                pss[:, 0:QT],
                                kpas[pb0:pb0 + 32, m, ks], qpas[pb0:pb0 + 32, m, qs],
                                start=False, stop=True, tile_position=(pb0, 0))
                            nc.tensor.matmul(
                                pss[:, QT:2 * QT],
                                kpas[pb1:pb1 + 32, m, ks], qpas[pb1:pb1 + 32, m, qs],
                                start=False, stop=True, tile_position=(pb1, 0))
                            aT = work3.tile([P, 2 * QT], BF, tag="aT")
                            nc.scalar.activation(aT[:], pss[:], AF.Exp, scale=0.125)
                            nc.tensor.matmul(
                                po0[:], vaug_r[:, kc, h0, :], aT[:, 0:QT],
                                start=(kc == 0), stop=(kc == 7))
                            nc.tensor.matmul(
                                po1[:], vaug_r[:, kc, h1, :], aT[:, QT:2 * QT],
                                start=(kc == 0), stop=(kc == 7))
                            if kc % 2 == 1:
                                yield
                        for po, h in ((po0, h0), (po1, h1)):
                            den = work.tile([HD + 1, QT], F32, tag="den")
                            nc.vector.reciprocal(
                                den[HD:HD + 1, :], po[HD:HD + 1, :])
                            dsc = dram_pool.tile([1, QT], F32, tag="dsc")
                            nc.sync.dma_start(out=dsc[:], in_=den[HD:HD + 1, :])
                            bc = work.tile([HD, QT], F32, tag="bc")
                            nc.sync.dma_start(
                                out=bc[:], in_=dsc[:].to_broadcast((HD, QT)))
                            pbase = HD * (h % 2)
                            nc.vector.tensor_tensor(
                                outT[pbase:pbase + HD, h // 2, qs],
                                po[0:HD, :], bc[:], op=AL.mult)
                        yield

            # ---------------- output projection + bias
            def proj_units():
                for ot in range(8):
                    os_ = slice(ot * P, (ot + 1) * P)
                    for qt in range(NQ):
                        qs = slice(qt * QT, (qt + 1) * QT)
                        pt = ps_mm_pool.tile([P, QT], F32, tag="ps_mm")
                        for cc in range(8):
                            nc.tensor.matmul(
                                pt[:], wp[:, cc, os_], outT[:, cc, qs],
                                start=(cc == 0), stop=(cc == 7))
                        ys = work.tile([P, QT], F32, tag="ys")
                        nc.vector.tensor_scalar_add(ys[:], pt[:], biasT[:, ot:ot + 1])
                        nc.sync.dma_start(out=out_e[os_, qs], in_=ys[:])
                        yield

            def run(gen):
                for _ in gen:
                    pass

            def weave(a, b, ra=3, rb=1):
                """Alternate ra units from a with rb units from b."""
                a, b = iter(a), iter(b)
                alive_a = alive_b = True
                while alive_a or alive_b:
                    for _ in range(ra):
                        if alive_a:
                            try:
                                next(a)
                            except StopIteration:
                                alive_a = False
                    for _ in range(rb):
                        if alive_b:
                            try:
                                next(b)
                            except StopIteration:
                                alive_b = False

            run(v_units())
            run(qkv_units(0))
            weave(attn_units(0), qkv_units(1))
            weave(attn_units(1), qkv_units(2))
            weave(attn_units(2), qkv_units(3))
            run(attn_units(3))
            run(proj_units())

    nc.compile()
    return nc


def _get_nc():
    global _BUILT
    if _BUILT is None:
        _BUILT = _build()
    return _BUILT


# ------------------------------------------------- tracing support (axon)

def _ensure_trace_hooks():
    """Register the NTFF profile hook that the bare agent image's antenv
    stub lacks, and neuter the artifact upload (no bucket in-container)."""
    import types
    import concourse.bass_utils as bu

    bu.upload_artifacts = lambda tmpdir: f"local:{tmpdir}"
    try:
        from antenv.axon_hooks import get_axon_ntff_profile_hook  # noqa: F401
        return
    except ImportError:
        pass
    mod = types.ModuleType("antenv.axon_hooks")
    _state = {"hook": None}
    mod.set_axon_ntff_profile_hook = lambda h: _state.__setitem__("hook", h)
    mod.get_axon_ntff_profile_hook = lambda: _state["hook"]
    import antenv
    sys.modules["antenv.axon_hooks"] = mod
    antenv.axon_hooks = mod
    try:
        from trn_agent_boot.trn_boot import _ntff_profile_via_ctypes
        hook = _ntff_profile_via_ctypes("/opt/axon/libaxon_pjrt.so")
        if hook is not None:
            mod.set_axon_ntff_profile_hook(hook)
    except Exception as e:  # pragma: no cover
        print(f"NTFF hook install failed: {e!r}")


# ----------------------------------------------------------------- kernel()

def kernel(x, Wqkv, Wproj, bproj):
    global LAST_RESULT
    x = np.asarray(x, np.float32)
    Wqkv = np.asarray(Wqkv, np.float32)
    Wproj = np.asarray(Wproj, np.float32)
    bproj = np.asarray(bproj, np.float32)
    B = x.shape[0]

    base = _prep_weights(Wqkv, Wproj, bproj)
    bf = ml_dtypes.bfloat16
    in_maps = [
        dict(base, xT=np.ascontiguousarray(x[b].T).astype(bf)) for b in range(B)
    ]
    nc = _get_nc()
    trace = bool(os.environ.get("KBENCH_TRACE"))
    if trace:
        _ensure_trace_hooks()
    res = run_bass_kernel_spmd(
        nc, in_maps, core_ids=list(range(B)), trace=trace)
    LAST_RESULT = res
    out = np.stack([np.asarray(res.results[b]["out"]).T for b in range(B)])
    return np.ascontiguousarray(out.astype(np.float32))


# revision 11
# speedup vs baseline: 1.2387x; 1.1815x over previous
"""8-core data-parallel fused attention kernel for TRN2 (Bass/Tile).

Problem: B=8, N=1024 (32x32 grid), DIM=1024, 16 heads x 64, axial RoPE on
first 32 channels of each head, softmax attention, output projection.

Sharding: pure data-parallel -- core b computes batch element b end-to-end.
No collectives.

Device layout tricks (all host-prepped):
- All matmuls in bf16 (PSUM accumulates f32). rel-err budget 2e-2.
- QKV computed "transposed" (features on partitions, tokens free) so that
  scores^T[k,q] matmuls need no on-chip transposes.
- rotate_half is folded into extra matmul columns: q~ = x @ (P Wq)^T where P
  is the pair-swap/negate permutation, applied to Wq rows host-side. RoPE is
  then elementwise: q_rope = q*cos + q~*sin (no partner-lane access).
- Q/K rot and pass channels live in separate SBUF tensors; scores contract
  d in two K=32 matmuls accumulated in PSUM, 2-way row-group packed.
- V gets a ones-column appended, so attn@V also yields softmax denominators
  (row 64 of the output) for free. exp() skips max-subtraction (scores are
  O(1) by construction: weights scaled 0.02).
- Softmax normalization: reciprocal on DVE + DMA partition-broadcast + one
  tensor_tensor multiply folded into the PSUM->SBUF drain.
"""

import os
import sys

for _p in ("/opt/trn_rl_repo",):
    if os.path.isdir(_p) and _p not in sys.path:
        sys.path.insert(0, _p)

import numpy as np
import ml_dtypes

import concourse.bass as bass
import concourse.bacc as bacc
import concourse.mybir as mybir
import concourse.tile as tile
from concourse.bass_utils import run_bass_kernel_spmd

P = 128
NTOK = 1024
DIM = 1024
HEADS = 16
HD = 64
ROT = 32
QT = 512          # free-dim tile for matmuls (one PSUM bank of f32)
NQ = NTOK // QT   # 2
BF = mybir.dt.bfloat16
F32 = mybir.dt.float32
AL = mybir.AluOpType
AF = mybir.ActivationFunctionType

LAST_RESULT = None
_BUILT = None


# ---------------------------------------------------------------- host prep

def _axial_tables():
    """cos/sin[t, d] for t=0..1023 (t=h*32+w), d=0..31, exactly as reference."""
    rot_half = 8
    base = np.linspace(1.0, 512.0, rot_half) * np.pi          # (8,)
    th = np.linspace(-1.0, 1.0, 32)[:, None] * base[None, :]  # (32, 8)
    fh = np.repeat(th, 2, axis=-1)                            # (32, 16)
    freqs = np.zeros((32, 32, ROT))
    freqs[:, :, :16] = fh[:, None, :]                         # H-axis channels
    freqs[:, :, 16:] = fh[None, :, :]                         # W-axis channels
    f = freqs.reshape(NTOK, ROT)
    return np.cos(f).astype(np.float32), np.sin(f).astype(np.float32)


def _tilde(W):
    """Row permutation+negation implementing rotate_half: rows h*64+d."""
    W4 = W.reshape(HEADS, HD // 2, 2, DIM)
    Wt = np.empty_like(W4)
    Wt[:, :, 0] = -W4[:, :, 1]
    Wt[:, :, 1] = W4[:, :, 0]
    return Wt.reshape(HEADS * HD, DIM)


def _sel(W, lo, hi):
    """Quad-major row select: for m(4) for hl(4) for d in [lo,hi)."""
    idx = [(4 * m + hl) * HD + d
           for m in range(4) for hl in range(4) for d in range(lo, hi)]
    return W[idx]


def _prep_weights(Wqkv, Wproj, bproj):
    Wq, Wk, Wv = Wqkv[0:DIM], Wqkv[DIM:2 * DIM], Wqkv[2 * DIM:3 * DIM]
    Wqt, Wkt = _tilde(Wq), _tilde(Wk)
    # column groups: 0 Qrot, 1 Q~rot, 2 Krot, 3 K~rot, 4 Qpass, 5 Kpass
    # quad-major: for each quad m, 6 groups of 128 rows
    # (Qrot, Q~rot, Krot, K~rot, Qpass, Kpass), each covering heads 4m..4m+3
    grp = [
        _sel(Wq, 0, ROT), _sel(Wqt, 0, ROT),
        _sel(Wk, 0, ROT), _sel(Wkt, 0, ROT),
        _sel(Wq, ROT, HD), _sel(Wk, ROT, HD),
    ]
    wqk = np.concatenate(
        [g[m * P:(m + 1) * P] for m in range(4) for g in grp], axis=0)
    cos_td, sin_td = _axial_tables()          # (1024, 32)
    cosR = np.tile(cos_td.T, (4, 1))          # (128, 1024): row hl*32+d
    sinR = np.tile(sin_td.T, (4, 1))
    biasT = bproj.reshape(8, P).T.copy()      # (128, 8)
    bf = ml_dtypes.bfloat16
    return {
        "wqk": np.ascontiguousarray(wqk.T).astype(bf),     # (1024, 3072)
        "wv": np.ascontiguousarray(Wv.T).astype(bf),       # (1024, 1024)
        "wp": np.ascontiguousarray(Wproj.T).astype(bf),    # (1024, 1024)
        "cosr": np.ascontiguousarray(cosR.astype(np.float32)),
        "sinr": np.ascontiguousarray(sinR.astype(np.float32)),
        "biasT": np.ascontiguousarray(biasT.astype(np.float32)),
    }


# ------------------------------------------------------------- bass builder

def _build():
    nc = bacc.Bacc()
    xT_e = nc.declare_dram_parameter("xT", [DIM, NTOK], BF, isOutput=False)
    wqk_e = nc.declare_dram_parameter("wqk", [DIM, 3 * DIM], BF, isOutput=False)
    wv_e = nc.declare_dram_parameter("wv", [DIM, DIM], BF, isOutput=False)
    wp_e = nc.declare_dram_parameter("wp", [DIM, DIM], BF, isOutput=False)
    cos_e = nc.declare_dram_parameter("cosr", [P, NTOK], F32, isOutput=False)
    sin_e = nc.declare_dram_parameter("sinr", [P, NTOK], F32, isOutput=False)
    b_e = nc.declare_dram_parameter("biasT", [P, 8], F32, isOutput=False)
    out_e = nc.declare_dram_parameter("out", [DIM, NTOK], F32, isOutput=True)

    with tile.TileContext(nc) as tc:
        with (
            tc.tile_pool(name="persist", bufs=1) as persist,
            tc.tile_pool(name="work", bufs=2) as work,
            tc.tile_pool(name="work3", bufs=3) as work3,
            tc.tile_pool(name="ps_sc", bufs=2, space="PSUM") as ps_sc_pool,
            tc.tile_pool(name="ps_av", bufs=2, space="PSUM") as ps_av_pool,
            tc.tile_pool(name="ps_mm", bufs=2, space="PSUM") as ps_mm_pool,
            tc.tile_pool(name="dramsc", bufs=4, space="DRAM") as dram_pool,
        ):
            xT = persist.tile([P, 8, NTOK], BF)
            wqk = persist.tile([P, 8, 3 * DIM], BF)
            wv = persist.tile([P, 8, DIM], BF)
            wp = persist.tile([P, 8, DIM], BF)
            cosr = persist.tile([P, NTOK], F32)
            sinr = persist.tile([P, NTOK], F32)
            biasT = persist.tile([P, 8], F32)
            # rope'd QK: partition hl*32+d, chunk = quad index m
            qrot = persist.tile([P, 4, NTOK], BF)
            qpas = persist.tile([P, 4, NTOK], BF)
            krot = persist.tile([P, 4, NTOK], BF)
            kpas = persist.tile([P, 4, NTOK], BF)
            # V with ones column: [k-token partitions, kc, head, 65]
            vaug = persist.tile([P, 8, HEADS * (HD + 1)], BF)
            # attention out, transposed: partition 64*(h%2)+d, chunk h//2
            outT = persist.tile([P, 8, NTOK], BF)

            vaug_r = vaug.rearrange("p n (h e) -> p n h e", e=HD + 1)

            # ---------------- input DMAs. Order matters: xT/wv feed the V
            # phase, wqk quad 0 feeds qkv(0); the rest streams in behind.
            for cc in range(8):
                nc.sync.dma_start(out=xT[:, cc, :], in_=xT_e[cc * P:(cc + 1) * P, :])
            for cc in range(8):
                nc.sync.dma_start(out=wv[:, cc, :], in_=wv_e[cc * P:(cc + 1) * P, :])
            nc.sync.dma_start(out=cosr[:], in_=cos_e[:, :])
            nc.sync.dma_start(out=sinr[:], in_=sin_e[:, :])
            nc.sync.dma_start(out=biasT[:], in_=b_e[:, :])
            # wqk columns are quad-major: quad m occupies cols [m*768, (m+1)*768)
            for m in range(4):
                for cc in range(8):
                    nc.sync.dma_start(
                        out=wqk[:, cc, m * 768:(m + 1) * 768],
                        in_=wqk_e[cc * P:(cc + 1) * P, m * 768:(m + 1) * 768])
            for cc in range(8):
                nc.sync.dma_start(out=wp[:, cc, :], in_=wp_e[cc * P:(cc + 1) * P, :])

            # ---------------- V = x @ Wv^T, natural orientation
            def v_units():
                for tt in range(8):
                    for g in range(2):
                        pt = ps_mm_pool.tile([P, QT], F32, tag="ps_mm")
                        for cc in range(8):
                            nc.tensor.matmul(
                                pt[:],
                                xT[:, cc, tt * P:(tt + 1) * P],
                                wv[:, cc, g * QT:(g + 1) * QT],
                                start=(cc == 0), stop=(cc == 7))
                        nc.vector.tensor_copy(
                            vaug_r[:, tt, g * 8:(g + 1) * 8, 0:HD],
                            pt[:].rearrange("p (h d) -> p h d", d=HD))
                        yield
                    nc.vector.memset(vaug_r[:, tt, :, HD:HD + 1], 1.0)

            # ---------------- QKV^T for one quad of 4 heads + RoPE epilogue
            def qkv_units(m):
                def mm_group(gi, t2):
                    pt = ps_mm_pool.tile([P, QT], F32, tag="ps_mm")
                    col0 = m * 768 + gi * P
                    ts_ = slice(t2 * QT, (t2 + 1) * QT)
                    for cc in range(8):
                        nc.tensor.matmul(
                            pt[:],
                            wqk[:, cc, col0:col0 + P],
                            xT[:, cc, ts_],
                            start=(cc == 0), stop=(cc == 7))
                    return pt

                for t2 in range(NQ):
                    ts_ = slice(t2 * QT, (t2 + 1) * QT)
                    for rot_g, til_g, dst in ((0, 1, qrot), (2, 3, krot)):
                        pr = mm_group(rot_g, t2)
                        yield
                        ptl = mm_group(til_g, t2)
                        t1 = work.tile([P, QT], BF, tag="t1")
                        t2b = work.tile([P, QT], BF, tag="t2")
                        nc.vector.tensor_tensor(t1[:], pr[:], cosr[:, ts_], op=AL.mult)
                        nc.vector.tensor_tensor(t2b[:], ptl[:], sinr[:, ts_], op=AL.mult)
                        nc.vector.tensor_add(dst[:, m, ts_], t1[:], t2b[:])
                        yield
                    for pas_g, dst in ((4, qpas), (5, kpas)):
                        pp = mm_group(pas_g, t2)
                        nc.vector.tensor_copy(dst[:, m, ts_], pp[:])
                        yield

            # ---------------- attention for one quad (2 head-pairs)
            def attn_units(m):
                for p2 in range(2):
                    h0 = 4 * m + 2 * p2
                    h1 = h0 + 1
                    pb0 = (2 * p2) * 32
                    pb1 = pb0 + 32
                    for qt in range(NQ):
                        qs = slice(qt * QT, (qt + 1) * QT)
                        po0 = ps_av_pool.tile([HD + 1, QT], F32, tag="ps_o")
                        po1 = ps_av_pool.tile([HD + 1, QT], F32, tag="ps_o")
                        for kc in range(8):
                            ks = slice(kc * P, (kc + 1) * P)
                            pss = ps_sc_pool.tile([P, 2 * QT], F32, tag="ps_s")
                            nc.tensor.matmul(
                                pss[:, 0:QT],
                                krot[pb0:pb0 + 32, m, ks], qrot[pb0:pb0 + 32, m, qs],
                                start=True, stop=False, tile_position=(pb0, 0))
                            nc.tensor.matmul(
                                pss[:, QT:2 * QT],
                                krot[pb1:pb1 + 32, m, ks], qrot[pb1:pb1 + 32, m, qs],
                                start=True, stop=False, tile_position=(pb1, 0))
                            nc.tensor.matmul(
                                pss[:, 0:QT],
                                kpas[pb0:pb0 + 32, m, ks], qpas[pb0:pb0 + 32, m, qs],
                                start=False, stop=True, tile_position=(pb0, 0))
                            nc.tensor.matmul(
                                pss[:, QT:2 * QT],
                                kpas[pb1:pb1 + 32, m, ks], qpas[pb1:pb1 + 32, m, qs],
                                start=False, stop=True, tile_position=(pb1, 0))
                            aT = work3.tile([P, 2 * QT], BF, tag="aT")
                            nc.scalar.activation(aT[:], pss[:], AF.Exp, scale=0.125)
                            nc.tensor.matmul(
                                po0[:], vaug_r[:, kc, h0, :], aT[:, 0:QT],
                                start=(kc == 0), stop=(kc == 7))
                            nc.tensor.matmul(
                                po1[:], vaug_r[:, kc, h1, :], aT[:, QT:2 * QT],
                                start=(kc == 0), stop=(kc == 7))
                            if kc % 2 == 1:
                                yield
                        for po, h in ((po0, h0), (po1, h1)):
                            den = work.tile([HD + 1, QT], F32, tag="den")
                            # raw denominator row PSUM->SBUF, DMA-broadcast it
                            # across 64 partitions, then fast reciprocal on the
                            # full [64, 512] tile (single-partition custom-DVE
                            # recip miscomputes; this shape is verified-good)
                            nc.vector.tensor_copy(
                                den[HD:HD + 1, :], po[HD:HD + 1, :])
                            dsc = dram_pool.tile([1, QT], F32, tag="dsc")
                            nc.sync.dma_start(out=dsc[:], in_=den[HD:HD + 1, :])
                            bc = work.tile([HD, QT], F32, tag="bc")
                            nc.sync.dma_start(
                                out=bc[:], in_=dsc[:].to_broadcast((HD, QT)))
                            bcr = work.tile([HD, QT], F32, tag="bcr")
                            nc.vector.reciprocal_approx_fast(bcr[:], bc[:])
                            pbase = HD * (h % 2)
                            nc.vector.tensor_tensor(
                                outT[pbase:pbase + HD, h // 2, qs],
                                po[0:HD, :], bcr[:], op=AL.mult)
                        yield

            # ---------------- output projection + bias
            def proj_units():
                for ot in range(8):
                    os_ = slice(ot * P, (ot + 1) * P)
                    for qt in range(NQ):
                        qs = slice(qt * QT, (qt + 1) * QT)
                        pt = ps_mm_pool.tile([P, QT], F32, tag="ps_mm")
                        for cc in range(8):
                            nc.tensor.matmul(
                                pt[:], wp[:, cc, os_], outT[:, cc, qs],
                                start=(cc == 0), stop=(cc == 7))
                        ys = work.tile([P, QT], F32, tag="ys")
                        nc.vector.tensor_scalar_add(ys[:], pt[:], biasT[:, ot:ot + 1])
                        nc.sync.dma_start(out=out_e[os_, qs], in_=ys[:])
                        yield

            def run(gen):
                for _ in gen:
                    pass

            def weave(a, b, ra=3, rb=1):
                """Alternate ra units from a with rb units from b."""
                a, b = iter(a), iter(b)
                alive_a = alive_b = True
                while alive_a or alive_b:
                    for _ in range(ra):
                        if alive_a:
                            try:
                                next(a)
                            except StopIteration:
                                alive_a = False
                    for _ in range(rb):
                        if alive_b:
                            try:
                                next(b)
                            except StopIteration:
                                alive_b = False

            run(v_units())
            run(qkv_units(0))
            weave(attn_units(0), qkv_units(1))
            weave(attn_units(1), qkv_units(2))
            weave(attn_units(2), qkv_units(3))
            run(attn_units(3))
            run(proj_units())

    nc.compile()
    return nc


def _get_nc():
    global _BUILT
    if _BUILT is None:
        _BUILT = _build()
    return _BUILT


# ------------------------------------------------- tracing support (axon)

def _ensure_trace_hooks():
    """Register the NTFF profile hook that the bare agent image's antenv
    stub lacks, and neuter the artifact upload (no bucket in-container)."""
    import types
    import concourse.bass_utils as bu

    bu.upload_artifacts = lambda tmpdir: f"local:{tmpdir}"
    try:
        from antenv.axon_hooks import get_axon_ntff_profile_hook  # noqa: F401
        return
    except ImportError:
        pass
    mod = types.ModuleType("antenv.axon_hooks")
    _state = {"hook": None}
    mod.set_axon_ntff_profile_hook = lambda h: _state.__setitem__("hook", h)
    mod.get_axon_ntff_profile_hook = lambda: _state["hook"]
    import antenv
    sys.modules["antenv.axon_hooks"] = mod
    antenv.axon_hooks = mod
    try:
        from trn_agent_boot.trn_boot import _ntff_profile_via_ctypes
        hook = _ntff_profile_via_ctypes("/opt/axon/libaxon_pjrt.so")
        if hook is not None:
            mod.set_axon_ntff_profile_hook(hook)
    except Exception as e:  # pragma: no cover
        print(f"NTFF hook install failed: {e!r}")


# ----------------------------------------------------------------- kernel()

def kernel(x, Wqkv, Wproj, bproj):
    global LAST_RESULT
    x = np.asarray(x, np.float32)
    Wqkv = np.asarray(Wqkv, np.float32)
    Wproj = np.asarray(Wproj, np.float32)
    bproj = np.asarray(bproj, np.float32)
    B = x.shape[0]

    base = _prep_weights(Wqkv, Wproj, bproj)
    bf = ml_dtypes.bfloat16
    in_maps = [
        dict(base, xT=np.ascontiguousarray(x[b].T).astype(bf)) for b in range(B)
    ]
    nc = _get_nc()
    trace = bool(os.environ.get("KBENCH_TRACE"))
    if trace:
        _ensure_trace_hooks()
    res = run_bass_kernel_spmd(
        nc, in_maps, core_ids=list(range(B)), trace=trace)
    LAST_RESULT = res
    out = np.stack([np.asarray(res.results[b]["out"]).T for b in range(B)])
    return np.ascontiguousarray(out.astype(np.float32))


# revision 13
# speedup vs baseline: 1.2576x; 1.0152x over previous
"""8-core data-parallel fused attention kernel for TRN2 (Bass/Tile).

Problem: B=8, N=1024 (32x32 grid), DIM=1024, 16 heads x 64, axial RoPE on
first 32 channels of each head, softmax attention, output projection.

Sharding: pure data-parallel -- core b computes batch element b end-to-end.
No collectives.

Device layout tricks (all host-prepped):
- All matmuls in bf16 (PSUM accumulates f32). rel-err budget 2e-2.
- QKV computed "transposed" (features on partitions, tokens free) so that
  scores^T[k,q] matmuls need no on-chip transposes.
- rotate_half is folded into extra matmul columns: q~ = x @ (P Wq)^T where P
  is the pair-swap/negate permutation, applied to Wq rows host-side. RoPE is
  then elementwise: q_rope = q*cos + q~*sin (no partner-lane access).
- Q/K rot and pass channels live in separate SBUF tensors; scores contract
  d in two K=32 matmuls accumulated in PSUM, 2-way row-group packed.
- V gets a ones-column appended, so attn@V also yields softmax denominators
  (row 64 of the output) for free. exp() skips max-subtraction (scores are
  O(1) by construction: weights scaled 0.02).
- Softmax normalization: reciprocal on DVE + DMA partition-broadcast + one
  tensor_tensor multiply folded into the PSUM->SBUF drain.
"""

import os
import sys

for _p in ("/opt/trn_rl_repo",):
    if os.path.isdir(_p) and _p not in sys.path:
        sys.path.insert(0, _p)

import numpy as np
import ml_dtypes

import concourse.bass as bass
import concourse.bacc as bacc
import concourse.mybir as mybir
import concourse.tile as tile
from concourse.bass_utils import run_bass_kernel_spmd

P = 128
NTOK = 1024
DIM = 1024
HEADS = 16
HD = 64
ROT = 32
QT = 512          # free-dim tile for matmuls (one PSUM bank of f32)
NQ = NTOK // QT   # 2
BF = mybir.dt.bfloat16
F32 = mybir.dt.float32
AL = mybir.AluOpType
AF = mybir.ActivationFunctionType

LAST_RESULT = None
_BUILT = None


# ---------------------------------------------------------------- host prep

def _axial_tables():
    """cos/sin[t, d] for t=0..1023 (t=h*32+w), d=0..31, exactly as reference."""
    rot_half = 8
    base = np.linspace(1.0, 512.0, rot_half) * np.pi          # (8,)
    th = np.linspace(-1.0, 1.0, 32)[:, None] * base[None, :]  # (32, 8)
    fh = np.repeat(th, 2, axis=-1)                            # (32, 16)
    freqs = np.zeros((32, 32, ROT))
    freqs[:, :, :16] = fh[:, None, :]                         # H-axis channels
    freqs[:, :, 16:] = fh[None, :, :]                         # W-axis channels
    f = freqs.reshape(NTOK, ROT)
    return np.cos(f).astype(np.float32), np.sin(f).astype(np.float32)


def _tilde(W):
    """Row permutation+negation implementing rotate_half: rows h*64+d."""
    W4 = W.reshape(HEADS, HD // 2, 2, DIM)
    Wt = np.empty_like(W4)
    Wt[:, :, 0] = -W4[:, :, 1]
    Wt[:, :, 1] = W4[:, :, 0]
    return Wt.reshape(HEADS * HD, DIM)


def _sel(W, lo, hi):
    """Quad-major row select: for m(4) for hl(4) for d in [lo,hi)."""
    idx = [(4 * m + hl) * HD + d
           for m in range(4) for hl in range(4) for d in range(lo, hi)]
    return W[idx]


def _prep_weights(Wqkv, Wproj, bproj):
    Wq, Wk, Wv = Wqkv[0:DIM], Wqkv[DIM:2 * DIM], Wqkv[2 * DIM:3 * DIM]
    Wqt, Wkt = _tilde(Wq), _tilde(Wk)
    # column groups: 0 Qrot, 1 Q~rot, 2 Krot, 3 K~rot, 4 Qpass, 5 Kpass
    # quad-major: for each quad m, 6 groups of 128 rows
    # (Qrot, Q~rot, Krot, K~rot, Qpass, Kpass), each covering heads 4m..4m+3
    grp = [
        _sel(Wq, 0, ROT), _sel(Wqt, 0, ROT),
        _sel(Wk, 0, ROT), _sel(Wkt, 0, ROT),
        _sel(Wq, ROT, HD), _sel(Wk, ROT, HD),
    ]
    wqk = np.concatenate(
        [g[m * P:(m + 1) * P] for m in range(4) for g in grp], axis=0)
    cos_td, sin_td = _axial_tables()          # (1024, 32)
    cosR = np.tile(cos_td.T, (4, 1))          # (128, 1024): row hl*32+d
    sinR = np.tile(sin_td.T, (4, 1))
    biasT = bproj.reshape(8, P).T.copy()      # (128, 8)
    bf = ml_dtypes.bfloat16
    return {
        "wqk": np.ascontiguousarray(wqk.T).astype(bf),     # (1024, 3072)
        "wv": np.ascontiguousarray(Wv.T).astype(bf),       # (1024, 1024)
        "wp": np.ascontiguousarray(Wproj.T).astype(bf),    # (1024, 1024)
        "cosr": np.ascontiguousarray(cosR.astype(np.float32)),
        "sinr": np.ascontiguousarray(sinR.astype(np.float32)),
        "biasT": np.ascontiguousarray(biasT.astype(np.float32)),
    }


# ------------------------------------------------------------- bass builder

def _build():
    nc = bacc.Bacc()
    xT_e = nc.declare_dram_parameter("xT", [DIM, NTOK], BF, isOutput=False)
    wqk_e = nc.declare_dram_parameter("wqk", [DIM, 3 * DIM], BF, isOutput=False)
    wv_e = nc.declare_dram_parameter("wv", [DIM, DIM], BF, isOutput=False)
    wp_e = nc.declare_dram_parameter("wp", [DIM, DIM], BF, isOutput=False)
    cos_e = nc.declare_dram_parameter("cosr", [P, NTOK], F32, isOutput=False)
    sin_e = nc.declare_dram_parameter("sinr", [P, NTOK], F32, isOutput=False)
    b_e = nc.declare_dram_parameter("biasT", [P, 8], F32, isOutput=False)
    out_e = nc.declare_dram_parameter("out", [DIM, NTOK], F32, isOutput=True)

    with tile.TileContext(nc) as tc:
        with (
            tc.tile_pool(name="persist", bufs=1) as persist,
            tc.tile_pool(name="work", bufs=2) as work,
            tc.tile_pool(name="work3", bufs=3) as work3,
            tc.tile_pool(name="ps_sc", bufs=2, space="PSUM") as ps_sc_pool,
            tc.tile_pool(name="ps_av", bufs=2, space="PSUM") as ps_av_pool,
            tc.tile_pool(name="ps_mm", bufs=2, space="PSUM") as ps_mm_pool,
            tc.tile_pool(name="dramsc", bufs=4, space="DRAM") as dram_pool,
        ):
            xT = persist.tile([P, 8, NTOK], BF)
            wqk = persist.tile([P, 8, 3 * DIM], BF)
            wv = persist.tile([P, 8, DIM], BF)
            wp = persist.tile([P, 8, DIM], BF)
            cosr = persist.tile([P, NTOK], F32)
            sinr = persist.tile([P, NTOK], F32)
            biasT = persist.tile([P, 8], F32)
            # rope'd QK: partition hl*32+d, chunk = quad index m
            qrot = persist.tile([P, 4, NTOK], BF)
            qpas = persist.tile([P, 4, NTOK], BF)
            krot = persist.tile([P, 4, NTOK], BF)
            kpas = persist.tile([P, 4, NTOK], BF)
            # V with ones column: [k-token partitions, kc, head, 65]
            vaug = persist.tile([P, 8, HEADS * (HD + 1)], BF)
            # attention out, transposed: partition 64*(h%2)+d, chunk h//2
            outT = persist.tile([P, 8, NTOK], BF)

            vaug_r = vaug.rearrange("p n (h e) -> p n h e", e=HD + 1)

            # ---------------- input DMAs. Order matters: xT/wv feed the V
            # phase, wqk quad 0 feeds qkv(0); the rest streams in behind.
            def load_wqk_quad(m):
                for cc in range(8):
                    nc.sync.dma_start(
                        out=wqk[:, cc, m * 768:(m + 1) * 768],
                        in_=wqk_e[cc * P:(cc + 1) * P, m * 768:(m + 1) * 768])

            for cc in range(8):
                nc.sync.dma_start(out=xT[:, cc, :], in_=xT_e[cc * P:(cc + 1) * P, :])
            load_wqk_quad(0)
            nc.sync.dma_start(out=cosr[:], in_=cos_e[:, :])
            nc.sync.dma_start(out=sinr[:], in_=sin_e[:, :])
            for cc in range(8):
                nc.sync.dma_start(out=wv[:, cc, :], in_=wv_e[cc * P:(cc + 1) * P, :])
            for m in range(1, 4):
                load_wqk_quad(m)
            nc.sync.dma_start(out=biasT[:], in_=b_e[:, :])
            for cc in range(8):
                nc.sync.dma_start(out=wp[:, cc, :], in_=wp_e[cc * P:(cc + 1) * P, :])

            # ---------------- V = x @ Wv^T, natural orientation
            def v_units():
                for tt in range(8):
                    for g in range(2):
                        pt = ps_mm_pool.tile([P, QT], F32, tag="ps_mm")
                        for cc in range(8):
                            nc.tensor.matmul(
                                pt[:],
                                xT[:, cc, tt * P:(tt + 1) * P],
                                wv[:, cc, g * QT:(g + 1) * QT],
                                start=(cc == 0), stop=(cc == 7))
                        nc.vector.tensor_copy(
                            vaug_r[:, tt, g * 8:(g + 1) * 8, 0:HD],
                            pt[:].rearrange("p (h d) -> p h d", d=HD))
                        yield
                    nc.vector.memset(vaug_r[:, tt, :, HD:HD + 1], 1.0)

            # ---------------- QKV^T for one quad of 4 heads + RoPE epilogue
            def qkv_units(m):
                def mm_group(gi, t2):
                    pt = ps_mm_pool.tile([P, QT], F32, tag="ps_mm")
                    col0 = m * 768 + gi * P
                    ts_ = slice(t2 * QT, (t2 + 1) * QT)
                    for cc in range(8):
                        nc.tensor.matmul(
                            pt[:],
                            wqk[:, cc, col0:col0 + P],
                            xT[:, cc, ts_],
                            start=(cc == 0), stop=(cc == 7))
                    return pt

                for t2 in range(NQ):
                    ts_ = slice(t2 * QT, (t2 + 1) * QT)
                    for rot_g, til_g, dst in ((0, 1, qrot), (2, 3, krot)):
                        pr = mm_group(rot_g, t2)
                        yield
                        ptl = mm_group(til_g, t2)
                        t1 = work.tile([P, QT], BF, tag="t1")
                        t2b = work.tile([P, QT], BF, tag="t2")
                        nc.vector.tensor_tensor(t1[:], pr[:], cosr[:, ts_], op=AL.mult)
                        nc.vector.tensor_tensor(t2b[:], ptl[:], sinr[:, ts_], op=AL.mult)
                        nc.vector.tensor_add(dst[:, m, ts_], t1[:], t2b[:])
                        yield
                    for pas_g, dst in ((4, qpas), (5, kpas)):
                        pp = mm_group(pas_g, t2)
                        nc.vector.tensor_copy(dst[:, m, ts_], pp[:])
                        yield

            # ---------------- attention for one quad (2 head-pairs)
            def attn_units(m, qt):
                for p2 in range(2):
                    h0 = 4 * m + 2 * p2
                    h1 = h0 + 1
                    pb0 = (2 * p2) * 32
                    pb1 = pb0 + 32
                    if True:
                        qs = slice(qt * QT, (qt + 1) * QT)
                        po0 = ps_av_pool.tile([HD + 1, QT], F32, tag="ps_o")
                        po1 = ps_av_pool.tile([HD + 1, QT], F32, tag="ps_o")
                        for kc in range(8):
                            ks = slice(kc * P, (kc + 1) * P)
                            pss = ps_sc_pool.tile([P, 2 * QT], F32, tag="ps_s")
                            nc.tensor.matmul(
                                pss[:, 0:QT],
                                krot[pb0:pb0 + 32, m, ks], qrot[pb0:pb0 + 32, m, qs],
                                start=True, stop=False, tile_position=(pb0, 0))
                            nc.tensor.matmul(
                                pss[:, QT:2 * QT],
                                krot[pb1:pb1 + 32, m, ks], qrot[pb1:pb1 + 32, m, qs],
                                start=True, stop=False, tile_position=(pb1, 0))
                            nc.tensor.matmul(
                                pss[:, 0:QT],
                                kpas[pb0:pb0 + 32, m, ks], qpas[pb0:pb0 + 32, m, qs],
                                start=False, stop=True, tile_position=(pb0, 0))
                            nc.tensor.matmul(
                                pss[:, QT:2 * QT],
                                kpas[pb1:pb1 + 32, m, ks], qpas[pb1:pb1 + 32, m, qs],
                                start=False, stop=True, tile_position=(pb1, 0))
                            aT = work3.tile([P, 2 * QT], BF, tag="aT")
                            nc.scalar.activation(aT[:], pss[:], AF.Exp, scale=0.125)
                            nc.tensor.matmul(
                                po0[:], vaug_r[:, kc, h0, :], aT[:, 0:QT],
                                start=(kc == 0), stop=(kc == 7))
                            nc.tensor.matmul(
                                po1[:], vaug_r[:, kc, h1, :], aT[:, QT:2 * QT],
                                start=(kc == 0), stop=(kc == 7))
                            if kc % 2 == 1:
                                yield
                        for po, h in ((po0, h0), (po1, h1)):
                            den = work.tile([HD + 1, QT], F32, tag="den")
                            # raw denominator row PSUM->SBUF, DMA-broadcast it
                            # across 64 partitions, then fast reciprocal on the
                            # full [64, 512] tile (single-partition custom-DVE
                            # recip miscomputes; this shape is verified-good)
                            nc.vector.tensor_copy(
                                den[HD:HD + 1, :], po[HD:HD + 1, :])
                            dsc = dram_pool.tile([1, QT], F32, tag="dsc")
                            nc.sync.dma_start(out=dsc[:], in_=den[HD:HD + 1, :])
                            bc = work.tile([HD, QT], F32, tag="bc")
                            nc.sync.dma_start(
                                out=bc[:], in_=dsc[:].to_broadcast((HD, QT)))
                            bcr = work.tile([HD, QT], F32, tag="bcr")
                            nc.vector.reciprocal_approx_fast(bcr[:], bc[:])
                            pbase = HD * (h % 2)
                            nc.vector.tensor_tensor(
                                outT[pbase:pbase + HD, h // 2, qs],
                                po[0:HD, :], bcr[:], op=AL.mult)
                        yield

            # ---------------- output projection + bias
            def proj_units(qt):
                for ot in range(8):
                    os_ = slice(ot * P, (ot + 1) * P)
                    if True:
                        qs = slice(qt * QT, (qt + 1) * QT)
                        pt = ps_mm_pool.tile([P, QT], F32, tag="ps_mm")
                        for cc in range(8):
                            nc.tensor.matmul(
                                pt[:], wp[:, cc, os_], outT[:, cc, qs],
                                start=(cc == 0), stop=(cc == 7))
                        ys = work.tile([P, QT], F32, tag="ys")
                        nc.vector.tensor_scalar_add(ys[:], pt[:], biasT[:, ot:ot + 1])
                        nc.sync.dma_start(out=out_e[os_, qs], in_=ys[:])
                        yield

            def run(gen):
                for _ in gen:
                    pass

            def weave(a, b, ra=3, rb=1):
                """Alternate ra units from a with rb units from b."""
                a, b = iter(a), iter(b)
                alive_a = alive_b = True
                while alive_a or alive_b:
                    for _ in range(ra):
                        if alive_a:
                            try:
                                next(a)
                            except StopIteration:
                                alive_a = False
                    for _ in range(rb):
                        if alive_b:
                            try:
                                next(b)
                            except StopIteration:
                                alive_b = False

            def chain(*gens):
                for g in gens:
                    for _ in g:
                        yield

            def attn_quad(m):
                return chain(attn_units(m, 0), attn_units(m, 1))

            run(qkv_units(0))
            run(v_units())
            weave(attn_quad(0), qkv_units(1), 3, 1)
            weave(attn_quad(1), qkv_units(2), 3, 1)
            weave(attn_quad(2), qkv_units(3), 3, 1)
            run(attn_units(3, 0))
            weave(attn_units(3, 1), proj_units(0), 2, 1)
            run(proj_units(1))

    nc.compile()
    return nc


def _get_nc():
    global _BUILT
    if _BUILT is None:
        _BUILT = _build()
    return _BUILT


# ------------------------------------------------- tracing support (axon)

def _ensure_trace_hooks():
    """Register the NTFF profile hook that the bare agent image's antenv
    stub lacks, and neuter the artifact upload (no bucket in-container)."""
    import types
    import concourse.bass_utils as bu

    bu.upload_artifacts = lambda tmpdir: f"local:{tmpdir}"
    try:
        from antenv.axon_hooks import get_axon_ntff_profile_hook  # noqa: F401
        return
    except ImportError:
        pass
    mod = types.ModuleType("antenv.axon_hooks")
    _state = {"hook": None}
    mod.set_axon_ntff_profile_hook = lambda h: _state.__setitem__("hook", h)
    mod.get_axon_ntff_profile_hook = lambda: _state["hook"]
    import antenv
    sys.modules["antenv.axon_hooks"] = mod
    antenv.axon_hooks = mod
    try:
        from trn_agent_boot.trn_boot import _ntff_profile_via_ctypes
        hook = _ntff_profile_via_ctypes("/opt/axon/libaxon_pjrt.so")
        if hook is not None:
            mod.set_axon_ntff_profile_hook(hook)
    except Exception as e:  # pragma: no cover
        print(f"NTFF hook install failed: {e!r}")


# ----------------------------------------------------------------- kernel()

def kernel(x, Wqkv, Wproj, bproj):
    global LAST_RESULT
    x = np.asarray(x, np.float32)
    Wqkv = np.asarray(Wqkv, np.float32)
    Wproj = np.asarray(Wproj, np.float32)
    bproj = np.asarray(bproj, np.float32)
    B = x.shape[0]

    base = _prep_weights(Wqkv, Wproj, bproj)
    bf = ml_dtypes.bfloat16
    in_maps = [
        dict(base, xT=np.ascontiguousarray(x[b].T).astype(bf)) for b in range(B)
    ]
    nc = _get_nc()
    trace = bool(os.environ.get("KBENCH_TRACE"))
    if trace:
        _ensure_trace_hooks()
    res = run_bass_kernel_spmd(
        nc, in_maps, core_ids=list(range(B)), trace=trace)
    LAST_RESULT = res
    out = np.stack([np.asarray(res.results[b]["out"]).T for b in range(B)])
    return np.ascontiguousarray(out.astype(np.float32))
